# revision 10
# baseline (speedup 1.0000x reference)
"""CrossAttentionFusion Trainium2 kernel.

Full inputs -> shard (batch x query-half) over 8 NeuronCores -> full output.

Per core (batch b = core//2, query half h = core%2, NH=2048 queries):
  Algebraic folding (host precompute):
    L[m,n] = K^T Q = x2^T (k_w^T q_w) x1 =: x2^T Q'   (K never materialized;
             terms constant in m cancel in softmax; x2^T k_w^T q_b folds
             into Q' channel bias)
    F_att   = v_w (x2 A_norm) + v_b  ->  M1 = (proj_w v_w) Z,  Z = x2 E
             (V never materialized; proj_w v_w and proj_w v_b precomputed)
  Device per 512-query block:
    L[m, n] = x2^T Q'                (bf16 matmuls, m on partitions)
    E = exp(L / 16)                  (ACT; no max subtraction: logits O(1))
    S[n] = sum_m E[m, n]             (2 DVE bf16 running chains + PE reduce)
    Z[c, n] = sum_m x2[c, m] E[m, n] (lhsT = host-pretransposed bf16 x2)
    M1 = P2 Z ;  out = x1 + gate * relu(M1 * G * (1/S) + Bc)
  with G = gamma*rsqrt(var+eps), Bc = beta + (proj_b + proj_w v_b - mean)*G.
  fusion(j-1) is interleaved into logits(j) on the PE; exp and the softmax
  sum run on ACT/DVE one step behind; 1/S is hidden under the next block.

Startup: a warmup activation hoists the ACT table load ahead of the input
DMA flood; x1/x2 stream in interleaved chunks so blk0 logits chase the DMA.
bf16 operands (x2r/x2t/Qt/E) halve input DMA and the DVE softmax chains;
numerically validated at ~4e-4 rel err (tolerance 2e-2).
"""
from contextlib import ExitStack

import numpy as np
import ml_dtypes

import concourse.bass as bass
import concourse.mybir as mybir
import concourse.tile as tile
from concourse import bacc
from concourse.bass_utils import run_bass_kernel_spmd

F32 = mybir.dt.float32
F32R = mybir.dt.float32r
BF16 = mybir.dt.bfloat16
AF = mybir.ActivationFunctionType
OP = mybir.AluOpType

B, C, H, W = 4, 256, 64, 64
N = H * W            # 4096
NCORES = 8
NH = N // 2          # 2048 queries per core
NBLK = 512           # query block
NBLOCKS = NH // NBLK
MT = N // 128        # 32 m-tiles
CH = 1024            # DMA chunk columns
EPS = 1e-5
SCALE = float(C) ** -0.5


def build():
    nc = bacc.Bacc("TRN2", target_bir_lowering=False, debug=False,
                   num_devices=NCORES)
    x1r_d = nc.dram_tensor("x1r", [C, NH], F32R, kind="ExternalInput")
    x2r_d = nc.dram_tensor("x2r", [C, N], BF16, kind="ExternalInput")
    x2t_d = nc.dram_tensor("x2t", [128, MT * C], BF16, kind="ExternalInput")
    wm_d = nc.dram_tensor("wmat", [C, 2 * C], F32R, kind="ExternalInput")
    cst_d = nc.dram_tensor("cst", [C, 6], F32, kind="ExternalInput")
    out_d = nc.dram_tensor("out", [C, NH], F32, kind="ExternalOutput")

    with tile.TileContext(nc) as tc, ExitStack() as ctx:
        pers = ctx.enter_context(tc.tile_pool(name="pers", bufs=1))
        work = ctx.enter_context(tc.tile_pool(name="work", bufs=2))
        psum = ctx.enter_context(tc.tile_pool(name="psum", bufs=1, space="PSUM"))

        # ---- persistent tiles ----
        wm = [pers.tile([128, 2 * C], F32R, tag=f"wm{ci}", name=f"wm{ci}") for ci in range(2)]
        cst = [pers.tile([128, 6], F32, tag=f"cst{ci}", name=f"cst{ci}") for ci in range(2)]
        gwr = [pers.tile([128, 2], F32R, tag=f"gwr{ci}", name=f"gwr{ci}") for ci in range(2)]
        gwb = [pers.tile([128, 1], BF16, tag=f"gwb{ci}", name=f"gwb{ci}") for ci in range(2)]
        x1r = [pers.tile([128, NH], F32R, tag=f"x1r{ci}", name=f"x1r{ci}") for ci in range(2)]
        x2r = [pers.tile([128, N], BF16, tag=f"x2r{ci}", name=f"x2r{ci}") for ci in range(2)]
        x2t = pers.tile([128, MT * C], BF16, tag="x2t", name="x2t")
        Qt = [pers.tile([128, NH], BF16, tag=f"Qt{co}", name=f"Qt{co}") for co in range(2)]
        grow = pers.tile([1, NH], F32R, tag="grow", name="grow")
        warm = pers.tile([1, 1], F32, tag="warm", name="warm")
        warm2 = pers.tile([1, 1], F32, tag="warm2", name="warm2")
        ones_f = pers.tile([128, 1], F32, tag="ones_f", name="ones_f")
        ones_f2 = pers.tile([1, 128], F32, tag="ones_f2", name="ones_f2")
        ones_cb = pers.tile([128, 1], BF16, tag="ones_cb", name="ones_cb")
        ones_k1 = pers.tile([1, 128], F32R, tag="ones_k1", name="ones_k1")
        E = pers.tile([128, MT * NBLK], BF16, tag="E", name="E")

        def fusion_mms(fp, mt):
            es = slice(mt * NBLK, (mt + 1) * NBLK)
            for co in range(2):
                nc.tensor.matmul(
                    fp[co][:], x2t[:, mt * C + co * 128: mt * C + (co + 1) * 128],
                    E[:, es], start=(mt == 0), stop=(mt == MT - 1))

        def sacc_adds(ca, cb, p):
            t0 = E[:, (2 * p) * NBLK:(2 * p + 1) * NBLK]
            t1 = E[:, (2 * p + 1) * NBLK:(2 * p + 2) * NBLK]
            if p == 0:
                nc.vector.tensor_add(ca[:], t0, t1)
            elif p == 1:
                nc.vector.tensor_add(cb[:], t0, t1)
            else:
                c = ca if p % 2 == 0 else cb
                nc.vector.tensor_add(c[:], c[:], t0)
                nc.vector.tensor_add(c[:], c[:], t1)

        def s_finalize(j, ca, cb):
            with nc.named_scope(f"sfin{j}"):
                sp = psum.tile([1, NBLK], F32, tag="s", name="s", bufs=1)
                nc.tensor.matmul(sp[:], ones_cb[:], ca[:], start=True, stop=False)
                nc.tensor.matmul(sp[:], ones_cb[:], cb[:], start=False, stop=True)
                invs_f = work.tile([1, NBLK], F32, tag="invs_f", name="invs_f",
                                   bufs=1)
                nc.vector.reciprocal_approx_fast(invs_f[:], sp[:])
                invs_r = work.tile([1, NBLK], F32R, tag="invs_r", name="invs_r",
                                   bufs=1)
                nc.vector.tensor_copy(invs_r[:], invs_f[:])
            return invs_r

        def post_block(j, fp, invs_r, nchunks=1, tail_dve=False):
            cw = NBLK // nchunks
            with nc.named_scope(f"post{j}"):
                Fs = [work.tile([128, NBLK], F32R, tag=f"Fs{co}", name=f"Fs{co}",
                                bufs=1) for co in range(2)]
                for co in range(2):
                    nc.scalar.activation(Fs[co][:], fp[co][:], AF.Copy)
                for ck in range(nchunks):
                    ns = slice(j * NBLK + ck * cw, j * NBLK + (ck + 1) * cw)
                    cs_f = slice(ck * cw, (ck + 1) * cw)
                    bc1 = psum.tile([128, cw], F32, tag="acc", name="acc", bufs=3)
                    nc.tensor.matmul(bc1[:], ones_k1[:], invs_r[:, cs_f])
                    invs_b = work.tile([128, cw], F32, tag="invs_b",
                                       name="invs_b", bufs=1)
                    nc.vector.tensor_copy(invs_b[:], bc1[:])
                    bc2 = psum.tile([128, cw], F32, tag="acc", name="acc", bufs=3)
                    nc.tensor.matmul(bc2[:], ones_k1[:], grow[:, ns])
                    gate_b = work.tile([128, cw], F32, tag="gate_b",
                                       name="gate_b", bufs=1)
                    nc.vector.tensor_copy(gate_b[:], bc2[:])
                    for co in range(2):
                        cs = slice(co * 128, (co + 1) * 128)
                        mp = psum.tile([128, cw], F32, tag="acc", name="acc",
                                       bufs=3)
                        for ci in range(2):
                            nc.tensor.matmul(
                                mp[:], wm[ci][:, C + co * 128: C + (co + 1) * 128],
                                Fs[ci][:, cs_f], start=(ci == 0), stop=(ci == 1))
                        t1 = work.tile([128, cw], F32, tag="t1", name="t1")
                        nc.vector.scalar_tensor_tensor(
                            t1[:], mp[:], cst[co][:, 1:2], invs_b[:],
                            op0=OP.mult, op1=OP.mult)
                        r = work.tile([128, cw], F32, tag="r", name="r")
                        nc.scalar.activation(r[:], t1[:], AF.Relu,
                                             bias=cst[co][:, 2:3])
                        rg = work.tile([128, cw], F32, tag="t1", name="rg")
                        ot = work.tile([128, cw], F32, tag="ot", name="ot")
                        if tail_dve:
                            nc.vector.tensor_mul(rg[:], r[:], gate_b[:])
                            nc.vector.tensor_add(ot[:], rg[:],
                                                 x1r[co][:, ns].bitcast(F32))
                        else:
                            nc.gpsimd.tensor_mul(rg[:], r[:], gate_b[:])
                            nc.gpsimd.tensor_add(ot[:], rg[:],
                                                 x1r[co][:, ns].bitcast(F32))
                        nc.sync.dma_start(out_d[cs, ns], ot[:])

        def emit_block(blk, prev_fp, ca, cb):
            ns = slice(blk * NBLK, (blk + 1) * NBLK)
            for mt2 in range(MT // 2):
                lp = psum.tile([128, 2 * NBLK], F32, tag="L", name="L", bufs=2)
                for sub in range(2):
                    mt = 2 * mt2 + sub
                    msl = slice(mt * 128, (mt + 1) * 128)
                    for ci in range(2):
                        nc.tensor.matmul(
                            lp[:, sub * NBLK:(sub + 1) * NBLK],
                            x2r[ci][:, msl], Qt[ci][:, ns],
                            start=(ci == 0), stop=(ci == 1))
                if prev_fp is not None:
                    fusion_mms(prev_fp, 2 * mt2)
                    fusion_mms(prev_fp, 2 * mt2 + 1)
                nc.scalar.activation(
                    E[:, mt2 * 2 * NBLK:(mt2 + 1) * 2 * NBLK], lp[:],
                    AF.Exp, scale=SCALE)
                if mt2 > 0:
                    sacc_adds(ca, cb, mt2 - 1)
            sacc_adds(ca, cb, MT // 2 - 1)

        def q_proj(nch):
            ns = slice(nch * NBLK, (nch + 1) * NBLK)
            for co in range(2):
                qp = psum.tile([128, NBLK], F32, tag="acc", name="acc", bufs=3)
                for ci in range(2):
                    nc.tensor.matmul(
                        qp[:], wm[ci][:, co * 128:(co + 1) * 128],
                        x1r[ci][:, ns], start=(ci == 0), stop=(ci == 1))
                nc.scalar.activation(Qt[co][:, ns], qp[:], AF.Identity,
                                     bias=cst[co][:, 0:1])

        with nc.named_scope("pre"):
            # warmup: hoist ACT table load before the input DMA flood
            nc.vector.memset(warm[:], 0.0)
            nc.scalar.activation(warm2[:], warm[:], AF.Exp)
            nc.vector.memset(ones_f[:], 1.0)
            nc.vector.tensor_copy(ones_cb[:], ones_f[:])
            nc.vector.memset(ones_f2[:], 1.0)
            nc.vector.tensor_copy(ones_k1[:], ones_f2[:])

            # DMA on 3 queues, urgency order. sync=ci0 rows, gpsimd=ci1 rows,
            # scalar(Act)=shared tail queue (x2r back half + all of x2t) so
            # blk0/blk1 operands never serialize behind one queue.
            qa = [nc.sync, nc.gpsimd]
            qb = nc.scalar
            for ci in range(2):
                cs = slice(ci * 128, (ci + 1) * 128)
                qa[ci].dma_start(wm[ci][:], wm_d[cs, :])
                qa[ci].dma_start(x1r[ci][:, 0:512], x1r_d[cs, 0:512])
                qa[ci].dma_start(x1r[ci][:, 512:CH], x1r_d[cs, 512:CH])
            for ch in range(4):
                chs = slice(ch * 512, (ch + 1) * 512)
                for ci in range(2):
                    cs = slice(ci * 128, (ci + 1) * 128)
                    qa[ci].dma_start(x2r[ci][:, chs], x2r_d[cs, chs])
            for ch in range(2, 4):
                chs = slice(ch * CH, (ch + 1) * CH)
                for ci in range(2):
                    cs = slice(ci * 128, (ci + 1) * 128)
                    qb.dma_start(x2r[ci][:, chs], x2r_d[cs, chs])
            for ci in range(2):
                cs = slice(ci * 128, (ci + 1) * 128)
                qa[ci].dma_start(cst[ci][:], cst_d[cs, :])
                qa[ci].dma_start(x1r[ci][:, CH:NH], x1r_d[cs, CH:NH])
            QT4 = MT * C // 4
            for sub in range(4):
                s0 = sub * QT4
                qb.dma_start(x2t[:, s0:s0 + QT4], x2t_d[:, s0:s0 + QT4])

            # gate weights rounded to f32r for the PE
            for ci in range(2):
                nc.vector.tensor_copy(gwr[ci][:], cst[ci][:, 3:5])
                nc.vector.tensor_copy(gwb[ci][:], cst[ci][:, 4:5])

            # Q' for blocks 0-1 (needs x1 chunk 0 only)
            q_proj(0)
            q_proj(1)

        sacc0 = (work.tile([128, NBLK], BF16, tag="sacca", name="sacca", bufs=2),
                 work.tile([128, NBLK], BF16, tag="saccb", name="saccb", bufs=2))
        with nc.named_scope("blk0"):
            emit_block(0, None, *sacc0)
        with nc.named_scope("pre2"):
            q_proj(2)
            q_proj(3)
        with nc.named_scope("gate"):
            # gate row (x2 columns pre-permuted: query pixels = 0..NH)
            for blk in range(NBLOCKS):
                ns = slice(blk * NBLK, (blk + 1) * NBLK)
                gp = psum.tile([1, NBLK], F32, tag="L", name="gp", bufs=2)
                for ci in range(2):
                    nc.tensor.matmul(gp[:], gwr[ci][:, 0:1], x1r[ci][:, ns],
                                     start=(ci == 0), stop=False)
                for ci in range(2):
                    nc.tensor.matmul(gp[:], gwb[ci][:], x2r[ci][:, ns],
                                     start=False, stop=(ci == 1))
                nc.scalar.activation(grow[:, ns], gp[:], AF.Sigmoid,
                                     bias=cst[0][0:1, 5:6])

        prev_fp = None
        prev_sacc = sacc0
        prev_invs = None
        prev = 0
        for blk in range(1, NBLOCKS):
            with nc.named_scope(f"blk{blk}"):
                prev_invs = s_finalize(prev, *prev_sacc)
                prev_fp = [psum.tile([128, NBLK], F32, tag="acc", name="acc",
                                     bufs=3) for _ in range(2)]
                sacc = (work.tile([128, NBLK], BF16, tag="sacca", name="sacca",
                                  bufs=2),
                        work.tile([128, NBLK], BF16, tag="saccb", name="saccb",
                                  bufs=2))
                emit_block(blk, prev_fp, *sacc)
            post_block(prev, prev_fp, prev_invs)
            prev = blk
            prev_sacc = sacc
        with nc.named_scope("tail"):
            prev_invs = s_finalize(prev, *prev_sacc)
            prev_fp = [psum.tile([128, NBLK], F32, tag="acc", name="acc", bufs=3)
                       for _ in range(2)]
            for mt in range(MT):
                fusion_mms(prev_fp, mt)
        post_block(prev, prev_fp, prev_invs, nchunks=2, tail_dve=True)
    nc.compile()
    return nc


_NC = None


def _get_nc():
    global _NC
    if _NC is None:
        _NC = build()
    return _NC


def kernel(**inputs):
    x1 = np.ascontiguousarray(np.asarray(inputs["x1"], dtype=np.float32)).reshape(B, C, N)
    x2 = np.ascontiguousarray(np.asarray(inputs["x2"], dtype=np.float32)).reshape(B, C, N)
    q_w = np.asarray(inputs["q_w"], np.float64)
    k_w = np.asarray(inputs["k_w"], np.float64)
    v_w = np.asarray(inputs["v_w"], np.float64)
    p_w = np.asarray(inputs["proj_w"], np.float64)
    q_b = np.asarray(inputs["q_b"], np.float64)
    v_b = np.asarray(inputs["v_b"], np.float64)
    p_b = np.asarray(inputs["proj_b"], np.float64)
    gamma = np.asarray(inputs["bn_gamma"], np.float64)
    beta = np.asarray(inputs["bn_beta"], np.float64)
    mean = np.asarray(inputs["bn_mean"], np.float64)
    var = np.asarray(inputs["bn_var"], np.float64)
    gate_w = np.asarray(inputs["gate_w"], np.float64)
    gate_b = np.asarray(inputs["gate_b"], np.float64)

    # folded weights: Q' = (k_w^T q_w) x1 + k_w^T q_b ;  M1 = (proj_w v_w) Z
    wqkT = (q_w.T @ k_w).astype(np.float32)          # lhsT for Q' projection
    p2T = (v_w.T @ p_w.T).astype(np.float32)         # lhsT for proj stage
    wmat = np.ascontiguousarray(np.concatenate([wqkT, p2T], axis=1))
    G = gamma / np.sqrt(var + EPS)
    Bc = beta + (p_b + p_w @ v_b - mean) * G
    qpb = k_w.T @ q_b
    gbcol = np.zeros(C)
    gbcol[0] = float(gate_b[0])
    cst = np.ascontiguousarray(
        np.stack([qpb, G, Bc, gate_w[0, :C], gate_w[0, C:], gbcol],
                 axis=1).astype(np.float32))

    in_maps = []
    for core in range(NCORES):
        b, half = divmod(core, 2)
        hq = slice(half * NH, (half + 1) * NH)
        ho = slice((1 - half) * NH, (2 - half) * NH)
        x1q = np.ascontiguousarray(x1[b][:, hq])
        x2p = np.ascontiguousarray(np.concatenate([x2[b][:, hq], x2[b][:, ho]],
                                                  axis=1))
        # x2 pretransposed into the fusion lhsT SBUF layout:
        # x2t[p, mt*C + c] = x2p[c, mt*128 + p]
        x2t = np.ascontiguousarray(
            x2p.reshape(C, MT, 128).transpose(2, 1, 0).reshape(128, MT * C)
            .astype(ml_dtypes.bfloat16))
        in_maps.append({
            "x1r": x1q,
            "x2r": np.ascontiguousarray(x2p.astype(ml_dtypes.bfloat16)),
            "x2t": x2t, "wmat": wmat, "cst": cst,
        })

    nc = _get_nc()
    res = run_bass_kernel_spmd(nc, in_maps, core_ids=list(range(NCORES)))
    out = np.empty((B, C, N), np.float32)
    for core in range(NCORES):
        b, half = divmod(core, 2)
        out[b, :, half * NH:(half + 1) * NH] = res.results[core]["out"]
    return out.reshape(B, C, H, W)
